# revision 11
# baseline (speedup 1.0000x reference)
"""Embedding gather-sum kernel for Trainium2 (8 NeuronCores, SPMD), v6.

Problem (nn_UserLinearUpscaler):
    out[b, s, :] = sum_k W[:, ids[b, s, k]] + bias
    B=1024, S=50, K=20, E=64, V=100000, f32 weights, integer ids.

Data-parallel over batch: each core handles 6400 tokens (128000 row
lookups).  W.T is stored in DRAM as bf16 rows padded to a 256 B stride
(wtp[v] = [row(v) bf16 (128 B) | pad]); the SWDGE gather fetches 128 B
elements on a 256 B stride, landing bf16 rows directly in SBUF.

Why this shape (measured on the target):
  * The SWDGE dma_gather is descriptor-rate bound: Pool-engine
    descriptor generation (~1.5 ns/row, serial across calls) and the
    per-queue ring drain (~7.6 ns/row/queue) cross over at 4 queues.
    4 balanced vocab ranges (25000 rows each) on 4 SWDGE queues run
    concurrently -> ~1.7 ns/row at 8 cores.
  * Large calls (T=1280-token chunks, ~6700 idxs/call, 20 calls/iter)
    amortize the ~1 us per-call fixed cost.
  * The stock wrapper asserts elem % 256 B == 0, but that restriction
    is transpose-only; a source-patched wrapper allows 128 B elements
    (verified correct on hardware), halving SBUF and HBM traffic.
  * Scatter: host packs each (chunk, range, psum-bank) slot list into
    128-slot blocks whose tokens fit a 128-token window.  VectorE
    builds S[128, blocks, 128] bf16 = is_equal(tok - w, iota) in one
    batched instruction per segment (per-instruction dispatch would
    otherwise dominate); PE accumulates psum[64, w:w+128] +=
    cg_blk.T @ S_blk (bf16).  PSUM has_written bits make sub-window
    accumulation safe without memset (first matmul of a bank uses
    start=True, which clears the whole bank).  ScalarE evicts
    psum + bias.

The block plan (window starts, block counts, DGE skip counts) is
derived from the actual ids and shared by all 8 cores (one NEFF), with
per-core pad spilling; packing is verified on the host.
"""

import inspect
import textwrap

import numpy as np

import concourse.bass as bass
import concourse.tile as tile
from concourse import bacc, mybir
from concourse.bass_utils import run_bass_kernel_spmd

B, S, K, E, V = 1024, 50, 20, 64, 100000
N_CORES = 8
P = 128
TOK_CORE = B // N_CORES * S          # 6400 tokens per core

T = 1280                             # tokens per gather chunk
CH = TOK_CORE // T                   # 5 chunks
NRANGE = 4
RSIZE = V // NRANGE                  # 25000 rows per vocab range
SUBS = (512, 512, 256)               # psum bank widths (tokens)
SUB_OFF = (0, 512, 1024)
NSUB = len(SUBS)
WIN = 128                            # token window per 128-slot block

DMA_SCRATCH = 32768

_cache: dict = {}


def _make_patched_gather():
    """dma_gather with the transpose-only 256 B element assert relaxed
    to 128 B (stride stays a multiple of 256 B, which the ISA encodes)."""
    src = inspect.getsource(bass.BassGpSimd.dma_gather)
    patched = src.replace(
        "assert (\n            elem_size_bytes > 0 and elem_size_bytes"
        " % 256 == 0\n        )  # transpose restriction",
        "assert elem_size_bytes > 0 and elem_size_bytes % 128 == 0")
    assert patched != src, "dma_gather assert rewrite failed"
    ns = dict(bass.__dict__)
    exec(textwrap.dedent(patched), ns)
    return ns["dma_gather"]


_dma_gather_128 = _make_patched_gather()


# --------------------------------------------------------------------------
# host-side planning / index building
# --------------------------------------------------------------------------

def _natural_segments(ids_core):
    """ids_core [TOK_CORE, K] int32 -> segs[(c, r, s)] = (tokens, locals)
    sorted by token (slot order within token preserved)."""
    segs = {}
    tok_of_slot = np.repeat(np.arange(TOK_CORE), K)
    flat = ids_core.reshape(-1)
    rng_id = flat // RSIZE
    local = flat - rng_id * RSIZE
    for c in range(CH):
        lo, hi = c * T * K, (c + 1) * T * K
        for r in range(NRANGE):
            sel = np.nonzero(rng_id[lo:hi] == r)[0]
            toks = tok_of_slot[lo + sel] - c * T      # chunk-local token
            locs = local[lo + sel]
            for s in range(NSUB):
                m = (toks >= SUB_OFF[s]) & (toks < SUB_OFF[s] + SUBS[s])
                segs[(c, r, s)] = (toks[m] - SUB_OFF[s], locs[m])
    return segs


def _make_plan(all_segs):
    """-> plan[(c, r, s)] = (nblk, [w_0..]); shared across cores."""
    plan = {}
    for c in range(CH):
        for r in range(NRANGE):
            for s in range(NSUB):
                key = (c, r, s)
                sub = SUBS[s]
                wmax = max(sub - WIN, 0)
                nblk = max(-(-len(segs[key][0]) // P) for segs in all_segs)
                nblk = max(nblk, 1)
                while True:
                    ws = []
                    for j in range(nblk):
                        w = min(
                            (segs[key][0][min(j * P,
                             max(len(segs[key][0]) - 1, 0))]
                             if len(segs[key][0]) else 0)
                            for segs in all_segs)
                        w = int(min(max(w - 4, 0), wmax)) & ~1
                        if ws:
                            w = max(w, ws[-1])            # monotone
                            w = min(w, ws[-1] + WIN - 28)  # step < WIN
                        ws.append(w)
                    ok = True
                    for segs in all_segs:
                        toks = segs[key][0]
                        j = used = 0
                        for t in toks:
                            while j < nblk and not (
                                    ws[j] <= t < ws[j] + WIN and used < P):
                                j += 1
                                used = 0
                            if j == nblk:
                                ok = False
                                break
                            used += 1
                        if not ok:
                            break
                    if ok:
                        break
                    nblk += 1
                plan[key] = (nblk, ws)
    return plan


def _pack_core(segs, plan):
    """-> out[(c, r)] = (gidx_flat, tokf_flat, last_real)."""
    out = {}
    for c in range(CH):
        for r in range(NRANGE):
            g_parts, t_parts = [], []
            last_real = 0
            pos = 0
            for s in range(NSUB):
                key = (c, r, s)
                nblk, ws = plan[key]
                toks, locs = segs[key]
                g = np.zeros(nblk * P, np.int16)          # pad idx 0
                tf = np.full(nblk * P, -1.0, np.float32)  # pad token -1
                j = used = 0
                for t, l in zip(toks, locs):
                    while not (ws[j] <= t < ws[j] + WIN and used < P):
                        j += 1
                        used = 0
                    g[j * P + used] = l
                    tf[j * P + used] = t - ws[j]
                    used += 1
                    last_real = pos + j * P + used
                g_parts.append(g)
                t_parts.append(tf)
                pos += nblk * P
            out[(c, r)] = (np.concatenate(g_parts),
                           np.concatenate(t_parts), last_real)
    return out


def _wrap16(flat):
    n = flat.shape[0]
    blk = flat.reshape(n // 16, 16).T
    return np.tile(blk, (8, 1))


def _build_inputs(per_core_ids):
    all_segs = [_natural_segments(ids) for ids in per_core_ids]
    plan = _make_plan(all_segs)
    packs = [_pack_core(segs, plan) for segs in all_segs]

    regs = {}
    for c in range(CH):
        for r in range(NRANGE):
            m = max(packs[i][(c, r)][2] for i in range(N_CORES))
            regs[(c, r)] = max(16, int(m))

    nr = {(c, r): sum(plan[(c, r, s)][0] * P for s in range(NSUB))
          for c in range(CH) for r in range(NRANGE)}
    nw_c = [sum(nr[(c, r)] for r in range(NRANGE)) // 16 for c in range(CH)]
    nb_c = [sum(plan[(c, r, s)][0] for r in range(NRANGE)
                for s in range(NSUB)) for c in range(CH)]
    NW, NB = max(nw_c), max(nb_c)

    ins = []
    for i in range(N_CORES):
        gidx = np.full((CH, P, NW), -1, np.int16)
        tokf = np.full((CH, P, NB), -1.0, np.float32)
        for c in range(CH):
            off_w = off_b = 0
            for r in range(NRANGE):
                g, tf, _ = packs[i][(c, r)]
                n = g.shape[0]
                gidx[c, :, off_w:off_w + n // 16] = _wrap16(g)
                tokf[c, :, off_b:off_b + n // P] = tf.reshape(n // P, P).T
                off_w += n // 16
                off_b += n // P
        ins.append({"gidx": gidx, "tokf": tokf})
    return plan, regs, ins, NW, NB


def _plan_key(plan, regs):
    return (tuple(sorted((k, v[0], tuple(v[1])) for k, v in plan.items())),
            tuple(sorted(regs.items())))


# --------------------------------------------------------------------------
# device kernel
# --------------------------------------------------------------------------

def _build(plan, regs, NW, NB, n_repeat=1):
    nc = bacc.Bacc("TRN2", target_bir_lowering=False, debug=False,
                   num_devices=N_CORES, num_swdge_queues=4,
                   dynamic_dma_scratch_size=DMA_SCRATCH)
    wt = nc.dram_tensor("wt", [V, 128], mybir.dt.bfloat16,
                        kind="ExternalInput")
    gidx = nc.dram_tensor("gidx", [CH, P, NW], mybir.dt.int16,
                          kind="ExternalInput")
    tokf = nc.dram_tensor("tokf", [CH, P, NB], mybir.dt.bfloat16,
                          kind="ExternalInput")
    iota = nc.dram_tensor("iota", [P, 1, WIN], mybir.dt.bfloat16,
                          kind="ExternalInput")
    biasc = nc.dram_tensor("biasc", [E, 1], mybir.dt.float32,
                           kind="ExternalInput")
    y = nc.dram_tensor("y", [CH, E, T], mybir.dt.float32,
                       kind="ExternalOutput")

    nr = {(c, r): sum(plan[(c, r, s)][0] * P for s in range(NSUB))
          for c in range(CH) for r in range(NRANGE)}

    with tile.TileContext(nc) as tc:
        with (
            tc.tile_pool(name="constp", bufs=1) as constp,
            tc.tile_pool(name="idxp", bufs=3) as idxp,
            tc.tile_pool(name="cgp", bufs=3) as cgp,
            tc.tile_pool(name="sp", bufs=6) as sp,
            tc.tile_pool(name="psump", bufs=2, space="PSUM") as psump,
            tc.tile_pool(name="evp", bufs=3) as evp,
        ):
            iota_t = constp.tile([P, 1, WIN], mybir.dt.bfloat16)
            nc.sync.dma_start(out=iota_t[:, :, :], in_=iota[:, :, :])
            biasc_t = constp.tile([E, 1], mybir.dt.float32)
            nc.sync.dma_start(out=biasc_t[:, :], in_=biasc[:, :])

            for _ in range(n_repeat):
                for c in range(CH):
                    gidx_t = idxp.tile([P, NW], mybir.dt.int16, tag="gidx")
                    nc.sync.dma_start(out=gidx_t[:, :], in_=gidx[c])
                    tokf_t = idxp.tile([P, NB, 1], mybir.dt.bfloat16,
                                       tag="tokf")
                    nc.sync.dma_start(out=tokf_t[:, :, :], in_=tokf[c])

                    cgs = []
                    off_w = 0
                    for r in range(NRANGE):
                        n_r = nr[(c, r)]
                        cg = cgp.tile([P, n_r // P, E], mybir.dt.bfloat16,
                                      tag=f"cg{r}")
                        _dma_gather_128(
                            nc.gpsimd,
                            out_ap=cg[:, :, :],
                            in_ap=wt[r * RSIZE:(r + 1) * RSIZE, 0:E],
                            idxs_ap=gidx_t[:, off_w:off_w + n_r // 16],
                            num_idxs=n_r,
                            num_idxs_reg=regs[(c, r)],
                            elem_size=E,
                            elem_step=128,
                            single_packet=False,
                            queue_num=r,
                        )
                        cgs.append(cg)
                        off_w += n_r // 16

                    blk_off, tok_off = {}, {}
                    for r in range(NRANGE):
                        o = 0
                        for s in range(NSUB):
                            blk_off[(r, s)] = o
                            o += plan[(c, r, s)][0]
                    o = 0
                    for r in range(NRANGE):
                        for s in range(NSUB):
                            tok_off[(r, s)] = o
                            o += plan[(c, r, s)][0]

                    for s in range(NSUB):
                        sub = SUBS[s]
                        psum = psump.tile([E, sub], mybir.dt.float32,
                                          tag=f"ps{s}")
                        n_mm = sum(plan[(c, r, s)][0] for r in range(NRANGE))
                        mm = 0
                        for r in range(NRANGE):
                            nblk, ws = plan[(c, r, s)]
                            col = tok_off[(r, s)]
                            s_t = sp.tile([P, nblk, WIN], mybir.dt.bfloat16,
                                          tag="S")
                            nc.vector.tensor_tensor(
                                out=s_t[:, :, :],
                                in0=tokf_t[:, col:col + nblk, :]
                                    .to_broadcast([P, nblk, WIN]),
                                in1=iota_t[:, :, :]
                                    .to_broadcast([P, nblk, WIN]),
                                op=mybir.AluOpType.is_equal)
                            for j in range(nblk):
                                w = ws[j]
                                nc.tensor.matmul(
                                    out=psum[:, w:w + WIN],
                                    lhsT=cgs[r][:, blk_off[(r, s)] + j, :],
                                    rhs=s_t[:, j, :],
                                    start=(mm == 0),
                                    stop=(mm == n_mm - 1),
                                    skip_group_check=True)
                                mm += 1

                        ev = evp.tile([E, sub], mybir.dt.float32, tag="ev")
                        nc.scalar.add(out=ev[:, :], in_=psum[:, :],
                                      add=biasc_t[:, 0:1])
                        nc.sync.dma_start(
                            out=y[c][:, SUB_OFF[s]:SUB_OFF[s] + sub],
                            in_=ev[:, :])
    nc.compile()
    return nc


# --------------------------------------------------------------------------
# entry point
# --------------------------------------------------------------------------

def _make_wtp(W):
    import ml_dtypes
    wt = W.T.astype(np.float32)                    # [V, E]
    wtp = np.zeros((V, 128), ml_dtypes.bfloat16)
    wtp[:, 0:E] = wt.astype(ml_dtypes.bfloat16)
    return np.ascontiguousarray(wtp)


def kernel(content_input: np.ndarray, W: np.ndarray, b: np.ndarray) -> np.ndarray:
    import ml_dtypes
    ids = np.ascontiguousarray(content_input).astype(np.int64) \
        .reshape(B * S, K).astype(np.int32)
    wtp = _make_wtp(W)
    iota = np.ascontiguousarray(
        np.arange(WIN, dtype=np.float32).reshape(1, 1, WIN)
        .repeat(P, axis=0).astype(ml_dtypes.bfloat16))
    biasc = np.ascontiguousarray(b.astype(np.float32).reshape(E, 1))

    per_core = [ids[i * TOK_CORE:(i + 1) * TOK_CORE] for i in range(N_CORES)]
    plan, regs, extra, NW, NB = _build_inputs(per_core)

    key = _plan_key(plan, regs)
    if key not in _cache:
        _cache[key] = _build(plan, regs, NW, NB)
    nc = _cache[key]

    in_maps = []
    for i in range(N_CORES):
        in_maps.append({"wt": wtp, "gidx": extra[i]["gidx"],
                        "tokf": extra[i]["tokf"].astype(ml_dtypes.bfloat16),
                        "iota": iota, "biasc": biasc})
    res = run_bass_kernel_spmd(nc, in_maps, core_ids=list(range(N_CORES)))
    out = np.concatenate(
        [res.results[i]["y"].transpose(0, 2, 1).reshape(TOK_CORE, E)
         for i in range(N_CORES)],
        axis=0)
    return out.reshape(B, S, E)
